# revision 71
# baseline (speedup 1.0000x reference)
"""Bahdanau (additive) attention kernel for Trainium2, 8 NeuronCores.

Problem: hidden [32,1,256], encoder_outputs [32,4096,256], W_attn [256,512],
b_attn [256], v [256]  ->  context [32,1,256]

  q_b        = hidden_b @ W1^T + b_attn                  (W1 = W_attn[:, :256])
  E_b[s,h]   = tanh(q_b[h] + sum_k enc_b[s,k] W2[h,k])   (W2 = W_attn[:, 256:])
  z_b[s]     = sum_h v[h] E_b[s,h]
  ctx_b[h]   = sum_s exp(z_s) enc_b[s,h] / sum_s exp(z_s)   (max|z| ~ 25, no
                                                             max-shift needed)

Sharding: data-parallel over batch, 4 batches per core, params replicated.

Device strategy (per core, per batch):
  - enc fed from HBM in TWO bf16 layouts (host-side prep): encT [k-part,
    s-free] for the energy matmul, encN [s-part, h-free] for context.
  - energy:  psum[E^T] = W2T(stationary) @ encT(moving), f32 accum.
  - tanh(+q bias) fused in ScalarE activation, PSUM->SBUF, output bf16.
  - scores:  z^T[128,32] psum via Et-block-stationary matmuls with v as the
    1-column moving operand -> scores land sequence-on-partitions.
  - exp on ScalarE with fused per-partition row-sum (accum_out) -> partZ.
  - ctx^T:   encN block [s-part, h-free] as STATIONARY, pT column as 1-col
    moving operand -> out [h-part, 1] accumulated over 32 s-blocks.  1-col
    matmuls make context nearly free on PE (vs M=1 row matmuls).
  - Zrep = ones[128,128] @ partZ replicates Z to all partitions so the final
    1/Z scale is a plain per-partition tensor_scalar on DVE.
  - output written as ctx^T [128, BL, KC] -> DRAM [BL, KC, 128].

bf16 enc quantization keeps end-to-end rel_err ~8e-3 (measured vs reference).
"""

import numpy as np
import ml_dtypes

B, S, H = 32, 4096, 256
NCORES = 8
BL = B // NCORES          # batches per core = 4
KC = H // 128             # feature chunks = 2
SB = S // 128             # sequence blocks = 32
NMM = 512                 # moving cols per matmul

_CACHE = {}


def _build_nc():
    import concourse.bass as bass
    import concourse.mybir as mybir
    from contextlib import ExitStack

    f32 = mybir.dt.float32
    bf16 = mybir.dt.bfloat16
    AF = mybir.ActivationFunctionType

    nc = bass.Bass()

    # qcst packs W1T | hidT | b_attn  ->  one DMA, one sem wait
    QW = H + BL + 1
    encT_d = nc.declare_dram_parameter("encT", [BL, KC, 128, S], bf16, isOutput=False)
    encN_d = nc.declare_dram_parameter("encN", [BL, SB, 128, H], bf16, isOutput=False)
    w2t_d = nc.declare_dram_parameter("w2t", [KC, 128, H], bf16, isOutput=False)
    qcst_d = nc.declare_dram_parameter("qcst", [KC, 128, QW], f32, isOutput=False)
    v_d = nc.declare_dram_parameter("vvec", [128, KC], bf16, isOutput=False)
    ones_d = nc.declare_dram_parameter("ones", [128, 128], f32, isOutput=False)
    # partition-major output; host transposes the tiny [128,BL,KC] result
    out_d = nc.declare_dram_parameter("out", [128, BL, KC], f32, isOutput=True)
    import os
    DEBUG = os.environ.get("KDEBUG", "0") == "1"
    if DEBUG:
        dbg_misc_d = nc.declare_dram_parameter("dbg_misc", [128, 8], f32, isOutput=True)
        dbg_zrep_d = nc.declare_dram_parameter("dbg_zrep", [128, 4], f32, isOutput=True)
        dbg_pt_d = nc.declare_dram_parameter("dbg_pt", [128, SB], f32, isOutput=True)
        dbg_pz_d = nc.declare_dram_parameter("dbg_pz", [128, 1], f32, isOutput=True)
        dbg_zt_d = nc.declare_dram_parameter("dbg_zt", [128, SB], f32, isOutput=True)
        dbg_en_d = nc.declare_dram_parameter("dbg_en", [128, SB, H], bf16, isOutput=True)
        dbg_pt4_d = nc.declare_dram_parameter("dbg_pt4", [128, BL, SB], f32, isOutput=True)
        dbg_q_d = nc.declare_dram_parameter("dbg_q", [128, KC, BL], f32, isOutput=True)

    NG = S // 1024            # s-groups per batch = 4 (each 1024 cols)
    NEG = KC * NG             # energy psum groups per batch = 8, i = g*KC+hc

    # ---- software-pipelined PE schedule ----
    # Energy groups + score chunks stream batch-major (encT arrives first in
    # the DMA order, so tanh never starves); Zrep/ctx run at the end, gated
    # by the trailing encN DMAs.  pT/partZ are per-batch so softmax state
    # never blocks the energy pipeline.
    pe_order = [("q", hc) for hc in range(KC)]
    for b in range(BL):
        sc = {3: [("sc", b, 0)], 5: [("sc", b, 1)], 7: [("sc", b, 2)]}
        for i in range(NEG):
            pe_order += [("en", b, i)]
            pe_order += sc.get(i, [])
        pe_order += [("sc", b, NG - 1)]
    for b in range(BL):
        pe_order += [("zr", b), ("cx", b)]

    act_order = [("qi", hc) for hc in range(KC)]
    for b in range(BL):
        act_order += [("th", b, i) for i in range(NEG)]
        act_order += [("ex", b)]
    peT = {k: i + 1 for i, k in enumerate(pe_order)}
    actT = {k: i + 1 for i, k in enumerate(act_order)}

    with ExitStack() as ctx:
        E = ctx.enter_context
        # SBUF
        w2t = E(nc.sbuf_tensor("w2t_s", [128, KC, H], bf16))
        qcst = E(nc.sbuf_tensor("qcst_s", [128, KC, QW], f32))
        vt = E(nc.sbuf_tensor("vt_s", [128, KC], bf16))
        ones_sb = E(nc.sbuf_tensor("ones_s", [128, 128], f32))
        q_sb = E(nc.sbuf_tensor("q_sb", [128, KC, BL], f32))
        encT = [E(nc.sbuf_tensor(f"encT{k}", [128, KC, S], bf16)) for k in range(3)]
        encN = [E(nc.sbuf_tensor(f"encN{k}", [128, SB, H], bf16)) for k in range(BL)]
        Et = [E(nc.sbuf_tensor(f"Et{k}", [128, KC, S], bf16)) for k in range(2)]
        pT = E(nc.sbuf_tensor("pT", [128, BL, SB], bf16))
        partZ = E(nc.sbuf_tensor("partZ", [128, BL], f32))
        rZrep = E(nc.sbuf_tensor("rZrep", [128, BL], f32))
        resT = E(nc.sbuf_tensor("resT", [128, BL, KC], f32))
        if DEBUG:
            dbg_misc = E(nc.sbuf_tensor("dbg_misc_s", [128, 8], f32))
            dbg_zrep = E(nc.sbuf_tensor("dbg_zrep_s", [128, 4], f32))
            dbg_pt = E(nc.sbuf_tensor("dbg_pt_s", [128, SB], f32))
            dbg_zt = E(nc.sbuf_tensor("dbg_zt_s", [128, SB], f32))
            dbg_pt4 = E(nc.sbuf_tensor("dbg_pt4_s", [128, BL, SB], f32))
        # PSUM (8 banks): e_ps 3x2 banks (3-deep energy/tanh pipe),
        # zq_ps 1 bank (zT cols 0:32 | q cols 32:40 | Zrep cols 40:44),
        # misc_ps 1 bank (ctx^T cols 2b:2b+2 per batch).
        NEPS = 3
        e_ps = [E(nc.psum_tensor(f"e_ps{k}", [128, 1024], f32))
                for k in range(NEPS)]
        zq_ps = E(nc.psum_tensor("zq_ps", [128, 512], f32))
        misc_ps = E(nc.psum_tensor("misc_ps", [128, 512], f32))
        zT_ps = zq_ps[:, 0:SB]
        # semaphores
        dmaC = E(nc.semaphore("dmaC"))
        dmaT = E(nc.semaphore("dmaT"))
        dmaN = E(nc.semaphore("dmaN"))
        dmaO = E(nc.semaphore("dmaO"))
        peS = E(nc.semaphore("peS"))
        actS = E(nc.semaphore("actS"))
        dveS = E(nc.semaphore("dveS"))
        blk = E(nc.Block())

        @blk.sync
        def _(sp):
            # encT in two 1MB chunks so energy starts on partial data;
            # encN trails (consumed only by the end-of-program ctx matmuls).
            SHALF = S // 2

            def dma_T(b, c):
                if b >= 3 and c == 0:         # encT slot reused by en(b-3)
                    sp.wait_ge(peS, peT[("en", b - 3, NEG - 1)])
                sp.dma_start(
                    out=encT[b % 3][:, :, c * SHALF:(c + 1) * SHALF],
                    in_=encT_d[b, :, :, c * SHALF:(c + 1) * SHALF]
                    .rearrange("c p s -> p c s")).then_inc(dmaT, 16)

            def dma_N(b):
                sp.dma_start(out=encN[b][:],
                             in_=encN_d[b].rearrange("j p h -> p j h")
                             ).then_inc(dmaN, 16)

            sp.dma_start(out=qcst[:], in_=qcst_d.rearrange("c p w -> p c w")
                         ).then_inc(dmaC, 16)
            sp.dma_start(out=w2t[:], in_=w2t_d.rearrange("c p h -> p c h")
                         ).then_inc(dmaC, 16)
            dma_T(0, 0)
            sp.dma_start(out=vt[:], in_=v_d[:]).then_inc(dmaC, 16)
            sp.dma_start(out=ones_sb[:], in_=ones_d[:]).then_inc(dmaC, 16)
            dma_T(0, 1)
            for b in range(1, BL):
                dma_T(b, 0); dma_T(b, 1)
            for b in range(BL):
                dma_N(b)
            sp.wait_ge(dveS, BL)
            sp.dma_start(out=out_d[:], in_=resT[:]).then_inc(dmaO, 16)
            if DEBUG:
                sp.dma_start(out=dbg_misc_d[:], in_=dbg_misc[:]).then_inc(dmaO, 16)
                sp.dma_start(out=dbg_zrep_d[:], in_=dbg_zrep[:]).then_inc(dmaO, 16)
                sp.dma_start(out=dbg_pt_d[:], in_=dbg_pt[:]).then_inc(dmaO, 16)
                sp.dma_start(out=dbg_pz_d[:], in_=partZ[:]).then_inc(dmaO, 16)
                sp.dma_start(out=dbg_zt_d[:], in_=dbg_zt[:]).then_inc(dmaO, 16)
                sp.dma_start(out=dbg_en_d[:], in_=encN[1][:]).then_inc(dmaO, 16)
                sp.dma_start(out=dbg_pt4_d[:], in_=dbg_pt4[:]).then_inc(dmaO, 16)
                sp.dma_start(out=dbg_q_d[:], in_=q_sb[:]).then_inc(dmaO, 16)
                sp.wait_ge(dmaO, 144)
            else:
                sp.wait_ge(dmaO, 16)

        @blk.tensor
        def _(pe):
            pe.wait_ge(dmaC, 32)
            for hc in range(KC):
                for fc in range(KC):
                    mm = pe.matmul(
                        zq_ps[:, 32 + hc * BL:32 + (hc + 1) * BL],
                        qcst[:, fc, hc * 128:(hc + 1) * 128],
                        qcst[:, fc, H:H + BL],
                        start=(fc == 0), stop=(fc == KC - 1))
                mm.then_inc(peS)
            for op in pe_order:
                if op[0] == "q":
                    continue                      # already emitted above
                if op[0] == "en":
                    _, b, i = op
                    g, hc = divmod(i, KC)
                    G = NEG * b + i
                    if i == 0 or (g == NG // 2 and hc == 0):
                        # encT chunk c=g//(NG//2) of batch b
                        pe.wait_ge(dmaT, 16 * (2 * b + g // (NG // 2) + 1))
                    if G >= NEPS:
                        pb, pi = divmod(G - NEPS, NEG)
                        pe.wait_ge(actS, actT[("th", pb, pi)])
                    for kc in range(KC):
                        for n in range(1024 // NMM):
                            mm = pe.matmul(
                                e_ps[G % NEPS][:, n * NMM:(n + 1) * NMM],
                                w2t[:, kc, hc * 128:(hc + 1) * 128],
                                encT[b % 3][:, kc,
                                            g * 1024 + n * NMM:
                                            g * 1024 + (n + 1) * NMM],
                                start=(kc == 0), stop=(kc == KC - 1))
                    mm.then_inc(peS)
                elif op[0] == "sc":
                    _, b, gq = op
                    if b == 0 and gq == 0:
                        pe.wait_ge(dmaC, 48)          # vt
                        pe.wait_ge(actS, actT[("qi", KC - 1)])
                    if b >= 1 and gq == 0:        # zT cols reused: ex(b-1) read
                        pe.wait_ge(actS, actT[("ex", b - 1)])
                    pe.wait_ge(actS, actT[("th", b, gq * KC + 1)])
                    for j in range(gq * 8, gq * 8 + 8):
                        for hc in range(KC):
                            mm = pe.matmul(
                                zT_ps[:, j:j + 1],
                                Et[b % 2][:, hc, j * 128:(j + 1) * 128],
                                vt[:, hc:hc + 1],
                                start=(hc == 0), stop=(hc == KC - 1))
                    mm.then_inc(peS)
                elif op[0] == "zr":
                    _, b = op
                    if b == 0:
                        pe.wait_ge(dmaC, 64)          # ones
                    pe.wait_ge(actS, actT[("ex", b)])
                    pe.matmul(zq_ps[:, 40 + b:41 + b], ones_sb[:],
                              partZ[:, b:b + 1], start=True,
                              stop=True).then_inc(peS)
                elif op[0] == "cx":
                    _, b = op
                    pe.wait_ge(dmaN, 16 * (b + 1))
                    pe.wait_ge(actS, actT[("ex", b)])
                    for hc in range(KC):
                        for j in range(SB):
                            mm = pe.matmul(
                                misc_ps[:, 2 * b + hc:2 * b + hc + 1],
                                encN[b][:, j, hc * 128:(hc + 1) * 128],
                                pT[:, b, j:j + 1],
                                start=(j == 0), stop=(j == SB - 1))
                    mm.then_inc(peS)

        @blk.scalar
        def _(act):
            act.wait_ge(dmaC, 32)
            for op in act_order:
                if op[0] == "qi":
                    _, hc = op
                    act.wait_ge(peS, peT[("q", hc)])
                    act.activation(q_sb[:, hc, :],
                                   zq_ps[:, 32 + hc * BL:32 + (hc + 1) * BL],
                                   AF.Identity,
                                   bias=qcst[:, hc, H + BL:H + BL + 1]
                                   ).then_inc(actS)
                elif op[0] == "th":
                    _, b, i = op
                    g, hc = divmod(i, KC)
                    if b >= 2 and i == 0:
                        act.wait_ge(peS, peT[("sc", b - 2, NG - 1)])
                    act.wait_ge(peS, peT[("en", b, i)])
                    act.activation(
                        Et[b % 2][:, hc, g * 1024:(g + 1) * 1024],
                        e_ps[(NEG * b + i) % NEPS][:],
                        AF.Tanh, bias=q_sb[:, hc, b:b + 1]).then_inc(actS)
                elif op[0] == "ex":
                    _, b = op
                    act.wait_ge(peS, peT[("sc", b, NG - 1)])
                    act.activation(pT[:, b, :], zT_ps[:], AF.Exp,
                                   accum_out=partZ[:, b:b + 1]).then_inc(actS)

        @blk.vector
        def _(dve):
            for b in range(BL):
                dve.wait_ge(peS, peT[("zr", b)])
                dve.reciprocal(rZrep[:, b:b + 1], zq_ps[:, 40 + b:41 + b])
                dve.drain()
                if DEBUG:
                    dve.tensor_copy(dbg_zrep[:, b:b + 1],
                                    zq_ps[:, 40 + b:41 + b])
                    dve.tensor_copy(dbg_pt4[:, b, :], pT[:, b, :])
                    if b == BL - 1:
                        dve.tensor_copy(dbg_pt[:], pT[:, b, :])
                        dve.tensor_copy(dbg_zt[:], zT_ps[:])
                dve.wait_ge(peS, peT[("cx", b)])
                if DEBUG:
                    dve.tensor_copy(dbg_misc[:, 2 * b:2 * b + 2],
                                    misc_ps[:, 2 * b:2 * b + 2])
                dve.tensor_scalar_mul(
                    resT[:, b, :], misc_ps[:, 2 * b:2 * b + 2],
                    rZrep[:, b:b + 1]).then_inc(dveS)

    return nc


def _get_nc():
    if "nc" not in _CACHE:
        _CACHE["nc"] = _build_nc()
    return _CACHE["nc"]


def _make_in_maps(hidden, encoder_outputs, W_attn, b_attn, v):
    bf16 = ml_dtypes.bfloat16
    hidden = np.asarray(hidden, dtype=np.float32)
    enc = np.asarray(encoder_outputs, dtype=np.float32)
    W_attn = np.asarray(W_attn, dtype=np.float32)
    b_attn = np.asarray(b_attn, dtype=np.float32)
    v = np.asarray(v, dtype=np.float32)

    w2t = np.ascontiguousarray(W_attn[:, H:].T).reshape(KC, 128, H).astype(bf16)
    vv = np.ascontiguousarray(v.reshape(KC, 128).T).astype(bf16)
    ones = np.ones((128, 128), dtype=np.float32)

    in_maps = []
    for i in range(NCORES):
        sl = slice(i * BL, (i + 1) * BL)
        enc_sh = enc[sl]                                    # [BL, S, H]
        encT = np.ascontiguousarray(enc_sh.transpose(0, 2, 1)).astype(bf16)
        encT = encT.reshape(BL, KC, 128, S)
        encN = np.ascontiguousarray(enc_sh).astype(bf16).reshape(BL, SB, 128, H)
        qcst = np.empty((KC, 128, H + BL + 1), dtype=np.float32)
        qcst[:, :, :H] = W_attn[:, :H].T.reshape(KC, 128, H)
        qcst[:, :, H:H + BL] = hidden[sl, 0, :].T.reshape(KC, 128, BL)
        qcst[:, :, H + BL] = b_attn.reshape(KC, 128)
        in_maps.append({
            "encT": encT, "encN": encN, "w2t": w2t, "qcst": qcst, "vvec": vv,
            "ones": ones,
        })
    return in_maps


def kernel(hidden, encoder_outputs, W_attn, b_attn, v):
    from concourse.bass_utils import run_bass_kernel_spmd

    nc = _get_nc()
    in_maps = _make_in_maps(hidden, encoder_outputs, W_attn, b_attn, v)
    res = run_bass_kernel_spmd(nc, in_maps, core_ids=list(range(NCORES)))
    outs = [np.asarray(res.results[i]["out"], dtype=np.float32)
            .transpose(1, 2, 0).reshape(BL, H)        # [128,BL,KC]->[BL,H]
            for i in range(NCORES)]
    ctx = np.concatenate(outs, axis=0)                      # [B, H]
    return ctx[:, None, :].astype(np.float32)
